# revision 30
# baseline (speedup 1.0000x reference)
"""Trainium2 Bass kernel for the DDS sampler problem.

Data-parallel over the batch axis: 8192 samples are split across 8
NeuronCores (1024 each). Each core runs the full 100-step sampler on its
shard; no cross-core communication.

Device data layout is feature-major ([feature, batch] in SBUF) so the
per-step MLP needs no transposes: the host pre-transposes the noise /
x0 / r shards during upload and re-transposes the trajectory output
during the gather.

The 1024-sample shard is split into two 512-column chunks that share the
128 SBUF partitions: chunk 0 lives on partitions 0:64, chunk 1 on 64:128
(z_dim = 64). MM1 runs per chunk (K = 128 r-rows + 64 x-rows, the x
matmuls placed via row tile_position so the two chunks' x-matmuls run
concurrently in the array), MM2 lands chunk 1's scores on partitions
64:128 via column tile_position, and from there every elementwise op
processes both chunks in one full-width instruction.

Matmul operands are bf16; the fp32 x-state is kept separately, with the
recurrence reassociated as x_new = sc2 + g, g = (1-cs)*x + sa*n
precomputed early so the post-MM2 critical chain is just two ops. The
per-step t-feature and biases are folded into per-step bias tables read
by the fused tanh/identity activations. A throwaway fp32 "heater"
matmul, dependency-pinned into the tail-chain window, keeps the PE's
HAM clock gate at 2.4 GHz across the per-step TensorE idle gap.
"""

import numpy as np

# Problem constants (hardcoded per the harness contract).
MB, NS, Z_DIM, R_DIM, HID, NUM_STEPS = 128, 64, 64, 128, 512, 100
B = MB * NS
T_TOTAL = 1.0
ETA = 1.0
DT = T_TOTAL / NUM_STEPS

N_CORES = 8
BS = B // N_CORES          # batch per core
NCH = 2                    # column chunks per core (share the 128 partitions)
CW = BS // NCH             # chunk width (512)

TRACE = False              # test harness can flip this for profiling
_LAST_RESULT = {}          # test harness introspection (exec_time_ns, trace)

_CACHE = {}


def _schedule(beta_max, beta_min):
    """Cosine-squared schedule constants, in float32 to match the reference."""
    f32 = np.float32
    softplus = lambda x: f32(np.logaddexp(f32(0.0), f32(x)))
    c_start = softplus(beta_max)
    c_end = softplus(beta_min)
    a = f32(c_start - c_end)
    b = f32(np.pi / (2.0 * T_TOTAL))
    c = c_end

    def F(t):
        t = f32(t)
        return f32(a * np.sin(f32(2.0 * b * t)) / (4.0 * b) + a * t / f32(2.0) + c * t)

    steps = np.arange(NUM_STEPS, dtype=np.float32)
    dF = np.array(
        [F((s + 1.0) * DT) - F(s * DT) for s in steps], dtype=np.float32
    )
    alpha = (f32(1.0) - np.exp(f32(-2.0) * dF)).astype(np.float32)
    kappa = ((ETA * (1.0 - np.sqrt(1.0 - alpha))) ** 2 / alpha).astype(np.float32)
    ts = (steps * f32(DT)).astype(np.float32)
    return ts, alpha, kappa


def _build(ts, alpha, kappa):
    """Build + compile the per-core Bass graph. Returns the Bacc object."""
    import concourse.bacc as bacc
    import concourse.mybir as mybir
    import concourse.tile as tile

    f32 = mybir.dt.float32
    bf16 = mybir.dt.bfloat16
    Alu = mybir.AluOpType
    Act = mybir.ActivationFunctionType

    nc = bacc.Bacc("TRN2", target_bir_lowering=False, debug=False,
                   num_devices=N_CORES)

    MT = HID // 128  # 4 m-tiles

    # DRAM parameters (per-core shards; weights replicated).
    noises_d = nc.dram_tensor("noises", [NUM_STEPS, 128, CW], f32,
                              kind="ExternalInput")
    x0_d = nc.dram_tensor("x0", [128, CW], f32, kind="ExternalInput")
    r_d = nc.dram_tensor("r", [NCH, R_DIM, CW], bf16, kind="ExternalInput")
    w1r_d = nc.dram_tensor("w1r", [R_DIM, HID], bf16, kind="ExternalInput")
    w1x2_d = nc.dram_tensor("w1x2", [128, HID], bf16, kind="ExternalInput")
    w2_d = nc.dram_tensor("w2", [128, MT, Z_DIM], bf16, kind="ExternalInput")
    b1t_d = nc.dram_tensor("b1t", [128, MT, NUM_STEPS], f32,
                           kind="ExternalInput")
    b2t_d = nc.dram_tensor("b2t", [128, NUM_STEPS], f32, kind="ExternalInput")
    muneg_d = nc.dram_tensor("muneg", [128, 1], f32, kind="ExternalInput")

    xt_d = nc.dram_tensor("xt", [NUM_STEPS + 1, 128, CW], f32,
                          kind="ExternalOutput")
    lw_d = nc.dram_tensor("lw", [NCH, CW], f32, kind="ExternalOutput")
    nab_d = nc.dram_tensor("nab", [128, CW], f32, kind="ExternalOutput")

    with tile.TileContext(nc) as tc:
        with (
            tc.tile_pool(name="const", bufs=1) as cpool,
            tc.tile_pool(name="xstp", bufs=3) as xpool,
            tc.tile_pool(name="xbfp", bufs=3) as xbpool,
            tc.tile_pool(name="noise", bufs=6) as npool,
            tc.tile_pool(name="work", bufs=3) as wpool,
            tc.tile_pool(name="hbuf", bufs=2) as hpool,
            tc.tile_pool(name="hps", bufs=3, space="PSUM") as hps_pool,
            tc.tile_pool(name="sps", bufs=1, space="PSUM") as sps_pool,
        ):
            # --- constants / weights ---
            w1r = cpool.tile([R_DIM, HID], bf16)
            nc.sync.dma_start(out=w1r[:], in_=w1r_d[:])
            w1x2 = cpool.tile([128, HID], bf16)
            nc.sync.dma_start(out=w1x2[:], in_=w1x2_d[:])
            w2 = cpool.tile([128, MT, Z_DIM], bf16)
            nc.sync.dma_start(out=w2[:], in_=w2_d[:])
            b1t = cpool.tile([128, MT, NUM_STEPS], f32)
            nc.sync.dma_start(out=b1t[:], in_=b1t_d[:])
            b2t = cpool.tile([128, NUM_STEPS], f32)
            nc.sync.dma_start(out=b2t[:], in_=b2t_d[:])
            muneg = cpool.tile([128, 1], f32)
            nc.sync.dma_start(out=muneg[:], in_=muneg_d[:])
            rbf = [cpool.tile([R_DIM, CW], bf16, name=f"rbf{c}")
                   for c in range(NCH)]
            for c in range(NCH):
                nc.sync.dma_start(out=rbf[c][:], in_=r_d[c])

            # --- x state: fp32 (xst) + bf16 matmul copy (xbf) ---
            xst = xpool.tile([128, CW], f32, tag="xst")
            nc.sync.dma_start(out=xst[:], in_=x0_d[:])
            xbf = xbpool.tile([128, CW], bf16, tag="xbf")
            nc.vector.tensor_copy(xbf[:], xst[:])
            nc.sync.dma_start(out=xt_d[0], in_=xst[:])

            # log-weight accumulator (both chunks merged on partitions)
            lwacc = cpool.tile([128, CW], f32, name="lwacc")
            nc.gpsimd.memset(lwacc[:], 0.0)

            # --- main loop ---
            for s in range(NUM_STEPS):
                cs = float(1.0 - np.sqrt(1.0 - alpha[s]))
                sa = float(ETA * np.sqrt(alpha[s]))
                cs1 = float(np.float32(1.0) - np.float32(cs))
                twosa = float(2.0 * np.sqrt(alpha[s]))

                n_t = npool.tile([128, CW], f32, tag="noise")
                nc.sync.dma_start(out=n_t[:], in_=noises_d[s])

                # g = (1-cs)*x + sa*n + 2cs*b2: all inputs exist at step
                # start, so this runs during MM1/tanh, off the critical
                # chain (b2t carries 2cs*b2 per partition).
                g_t = wpool.tile([128, CW], f32, tag="g")
                nc.vector.tensor_scalar(g_t[:], n_t[:], sa, b2t[:, s:s + 1],
                                        Alu.mult, Alu.add)
                nc.vector.scalar_tensor_tensor(g_t[:], xst[:], cs1, g_t[:],
                                               Alu.mult, Alu.add)

                # the r-half of MM1 doesn't depend on x: issue it first so the
                # PE has ready work while step s-1's tail chain completes
                # (keeps the HAM clock gate warm).
                h = hpool.tile([128, MT, 2 * CW], bf16, tag="h")
                pss = []
                for m in range(MT):
                    ms = slice(m * 128, (m + 1) * 128)
                    ps = hps_pool.tile([128, 2 * CW], f32, tag="hps")
                    pss.append(ps)
                    nc.tensor.matmul(ps[:, 0:CW], w1r[:, ms], rbf[0][:],
                                     start=True, stop=False)
                    nc.tensor.matmul(ps[:, CW:2 * CW], w1r[:, ms], rbf[1][:],
                                     start=True, stop=False)

                for m in range(MT):
                    ms = slice(m * 128, (m + 1) * 128)
                    ps = pss[m]
                    nc.tensor.matmul(ps[:, 0:CW], w1x2[0:64, ms],
                                     xbf[0:64, :], start=False, stop=True)
                    nc.tensor.matmul(ps[:, CW:2 * CW], w1x2[64:128, ms],
                                     xbf[64:128, :], start=False, stop=True)
                    # tanh(h_pre + b1 + t_s * W1[192]) fused via bias table
                    nc.scalar.activation(h[:, m, :], ps[:], Act.Tanh,
                                         bias=b1t[:, m, s:s + 1], scale=1.0)

                # MM2: chunk 0 -> partitions 0:64, chunk 1 -> 64:128
                sps = sps_pool.tile([128, CW], f32, tag="sps")
                for k in range(MT):
                    nc.tensor.matmul(sps[0:64, :], w2[:, k, :],
                                     h[:, k, 0:CW],
                                     start=(k == 0), stop=(k == MT - 1))
                    nc.tensor.matmul(sps[64:128, :], w2[:, k, :],
                                     h[:, k, CW:2 * CW],
                                     start=(k == 0), stop=(k == MT - 1))
                # x_new = 2cs*score + g, read straight from the MM2 PSUM:
                # ONE VectorE op on the critical path (bf16 out for next
                # step's x-matmuls); the fp32 twin runs off-path.
                xbf = xbpool.tile([128, CW], bf16, tag="xbf")
                nc.vector.scalar_tensor_tensor(xbf[:], sps[:], 2.0 * cs,
                                               g_t[:], Alu.mult, Alu.add)
                xst = xpool.tile([128, CW], f32, tag="xst")
                nc.vector.scalar_tensor_tensor(xst[:], sps[:], 2.0 * cs,
                                               g_t[:], Alu.mult, Alu.add)
                nc.sync.dma_start(out=xt_d[s + 1], in_=xst[:])

                # sc2 = 2*cs*(score + b2) on ScalarE (idle once the step's
                # tanh burst is done) — feeds only the log-weight ops now
                sc2 = wpool.tile([128, CW], f32, tag="sc")
                nc.scalar.activation(sc2[:], sps[:], Act.Identity,
                                     bias=b2t[:, s:s + 1], scale=2.0 * cs)

                # PE "heater": a throwaway fp32 matmul pinned (via its sc2
                # read) into the tail-chain window, so the HAM clock gate
                # never sees a long TensorE idle and the array stays at
                # 2.4 GHz. Output is never read.
                ht1 = sps_pool.tile([1, CW], f32, tag="lwps")
                nc.tensor.matmul(ht1[:], muneg[:, 0:1], sc2[:],
                                 start=True, stop=True)

                # log-weight increment (off the critical path):
                #   -2ka*s^2 - 2sqrt(ka)*s*n == -1/(2a) * sc2*(sc2+2sqrt(a)n)
                nka = float(-1.0 / (2.0 * alpha[s]))
                w_t = wpool.tile([128, CW], f32, tag="w")
                nc.vector.scalar_tensor_tensor(w_t[:], n_t[:], twosa,
                                               sc2[:], Alu.mult, Alu.add)
                p_t = wpool.tile([128, CW], f32, tag="p")
                nc.vector.tensor_tensor(p_t[:], sc2[:], w_t[:], Alu.mult)
                nc.vector.scalar_tensor_tensor(lwacc[:], p_t[:], nka,
                                               lwacc[:], Alu.mult, Alu.add)

            # --- epilogue: log_weights and nabla_g ---
            # terminal log-weight 0.5*(x^2 - (x-mu)^2), then reduce over the
            # 64 feature rows of each chunk via a ones-matmul (redtab[:, S]
            # holds the 0.5).
            xm = wpool.tile([128, CW], f32, tag="w")
            nc.scalar.activation(xm[:], xst[:], Act.Square,
                                 bias=muneg[:, 0:1], scale=1.0)
            xsq = wpool.tile([128, CW], f32, tag="m")
            nc.scalar.activation(xsq[:], xst[:], Act.Square, bias=0.0,
                                 scale=1.0)
            td = wpool.tile([128, CW], f32, tag="p")
            nc.vector.tensor_tensor(td[:], xsq[:], xm[:], Alu.subtract)
            tot = wpool.tile([128, CW], f32, tag="sc")
            nc.vector.scalar_tensor_tensor(tot[:], td[:], 0.5, lwacc[:],
                                           Alu.mult, Alu.add)
            ones = wpool.tile([128, 1], f32, tag="ones")
            nc.gpsimd.memset(ones[:], 1.0)
            for c in range(NCH):
                lw_ps = sps_pool.tile([1, CW], f32, tag="lwps")
                nc.tensor.matmul(lw_ps[:], ones[c * 64:(c + 1) * 64, 0:1],
                                 tot[c * 64:(c + 1) * 64, :],
                                 start=True, stop=True)
                lw_sb = wpool.tile([1, CW], f32, tag="lwsb")
                nc.vector.tensor_copy(lw_sb[:], lw_ps[:])
                nc.sync.dma_start(out=lw_d[c:c + 1, :], in_=lw_sb[:])

            # nabla_g = (x - mu) - x  (== -mu up to rounding, as reference)
            d1 = wpool.tile([128, CW], f32, tag="m")
            nc.scalar.activation(d1[:], xst[:], Act.Identity,
                                 bias=muneg[:, 0:1], scale=1.0)
            nab = wpool.tile([128, CW], f32, tag="sc")
            nc.vector.tensor_tensor(nab[:], d1[:], xst[:], Alu.subtract)
            nc.sync.dma_start(out=nab_d[:], in_=nab[:])

    nc.compile()
    return nc


def kernel(r, noises, x0, W1, b1, W2, b2, beta_max, beta_min, target_mu):
    import ml_dtypes
    from concourse.bass_utils import run_bass_kernel_spmd

    bf = ml_dtypes.bfloat16
    r = np.asarray(r, np.float32)
    noises = np.asarray(noises, np.float32)
    x0 = np.asarray(x0, np.float32)
    W1 = np.asarray(W1, np.float32)
    b1 = np.asarray(b1, np.float32)
    W2 = np.asarray(W2, np.float32)
    b2 = np.asarray(b2, np.float32)
    bmax = float(np.asarray(beta_max))
    bmin = float(np.asarray(beta_min))
    mu = np.asarray(target_mu, np.float32)

    ts, alpha, kappa = _schedule(bmax, bmin)

    key = (bmax, bmin)
    if key not in _CACHE:
        _CACHE[key] = _build(ts, alpha, kappa)
    nc = _CACHE[key]

    # Host-side shard prep (pure data movement + the schedule-bias folds).
    r_flat = r.reshape(B, R_DIM)
    # bias table: b1 + t_s * W1[last row] -> [128, 4, S]
    b1t = (b1[None, :] + ts[:, None] * W1[Z_DIM + R_DIM, :]).astype(np.float32)
    b1t = np.ascontiguousarray(
        b1t.reshape(NUM_STEPS, HID // 128, 128).transpose(2, 1, 0))
    cs_all = (1.0 - np.sqrt(1.0 - alpha)).astype(np.float32)  # [S]
    b2t_half = (2.0 * cs_all[None, :] * b2[:, None]).astype(np.float32)
    b2t_in = np.ascontiguousarray(np.concatenate([b2t_half, b2t_half], 0))
    muneg_in = np.ascontiguousarray(
        np.concatenate([-mu, -mu], 0)[:, None].astype(np.float32))
    w1r_in = np.ascontiguousarray(W1[Z_DIM:Z_DIM + R_DIM]).astype(bf)
    w1x2_in = np.ascontiguousarray(
        np.concatenate([W1[:Z_DIM], W1[:Z_DIM]], 0)).astype(bf)
    w2_in = np.ascontiguousarray(
        W2.reshape(HID // 128, 128, Z_DIM).transpose(1, 0, 2)).astype(bf)

    in_maps = []
    for i in range(N_CORES):
        b0, b1i = i * BS, (i + 1) * BS
        nsh = noises[b0:b1i]            # [BS, S, D]
        nT = np.ascontiguousarray(
            nsh.reshape(NCH, CW, NUM_STEPS, Z_DIM).transpose(2, 0, 3, 1)
        ).reshape(NUM_STEPS, 128, CW)
        x0T = np.ascontiguousarray(
            x0[b0:b1i].reshape(NCH, CW, Z_DIM).transpose(0, 2, 1)
        ).reshape(128, CW)
        rT = np.ascontiguousarray(
            r_flat[b0:b1i].reshape(NCH, CW, R_DIM).transpose(0, 2, 1)
        ).astype(bf)
        in_maps.append({
            "noises": nT, "x0": x0T, "r": rT,
            "w1r": w1r_in, "w1x2": w1x2_in, "w2": w2_in, "b1t": b1t,
            "b2t": b2t_in, "muneg": muneg_in,
        })

    res = run_bass_kernel_spmd(nc, in_maps, core_ids=list(range(N_CORES)),
                               trace=TRACE)
    _LAST_RESULT.clear()
    _LAST_RESULT["exec_time_ns"] = res.exec_time_ns
    _LAST_RESULT["trace"] = (res.instructions_and_trace[1]
                             if res.instructions_and_trace else None)
    _LAST_RESULT["profile_json"] = res.profile_json

    x_t = np.empty((B, NUM_STEPS + 1, Z_DIM), np.float32)
    log_w = np.empty((B,), np.float32)
    nab = np.empty((B, Z_DIM), np.float32)
    for i in range(N_CORES):
        b0, b1i = i * BS, (i + 1) * BS
        out = res.results[i]
        # [S+1, 128, CW] -> [NCH, CW, S+1, D]
        x_t[b0:b1i] = out["xt"].reshape(
            NUM_STEPS + 1, NCH, Z_DIM, CW).transpose(1, 3, 0, 2).reshape(
            BS, NUM_STEPS + 1, Z_DIM)
        log_w[b0:b1i] = out["lw"].reshape(BS)
        nab[b0:b1i] = out["nab"].reshape(NCH, Z_DIM, CW).transpose(
            0, 2, 1).reshape(BS, Z_DIM)
    return x_t, log_w, nab


# revision 31
# speedup vs baseline: 1.0172x; 1.0172x over previous
"""Trainium2 Bass kernel for the DDS sampler problem.

Data-parallel over the batch axis: 8192 samples are split across 8
NeuronCores (1024 each). Each core runs the full 100-step sampler on its
shard; no cross-core communication.

Device data layout is feature-major ([feature, batch] in SBUF) so the
per-step MLP needs no transposes: the host pre-transposes the noise /
x0 / r shards during upload and re-transposes the trajectory output
during the gather.

The 1024-sample shard is split into two 512-column chunks that share the
128 SBUF partitions: chunk 0 lives on partitions 0:64, chunk 1 on 64:128
(z_dim = 64). MM1 runs per chunk (K = 128 r-rows + 64 x-rows, the x
matmuls placed via row tile_position so the two chunks' x-matmuls run
concurrently in the array), MM2 lands chunk 1's scores on partitions
64:128 via column tile_position, and from there every elementwise op
processes both chunks in one full-width instruction.

Matmul operands are bf16; the fp32 x-state is kept separately, with the
recurrence reassociated as x_new = sc2 + g, g = (1-cs)*x + sa*n
precomputed early so the post-MM2 critical chain is just two ops. The
per-step t-feature and biases are folded into per-step bias tables read
by the fused tanh/identity activations. A throwaway fp32 "heater"
matmul, dependency-pinned into the tail-chain window, keeps the PE's
HAM clock gate at 2.4 GHz across the per-step TensorE idle gap.
"""

import numpy as np

# Problem constants (hardcoded per the harness contract).
MB, NS, Z_DIM, R_DIM, HID, NUM_STEPS = 128, 64, 64, 128, 512, 100
B = MB * NS
T_TOTAL = 1.0
ETA = 1.0
DT = T_TOTAL / NUM_STEPS

N_CORES = 8
BS = B // N_CORES          # batch per core
NCH = 2                    # column chunks per core (share the 128 partitions)
CW = BS // NCH             # chunk width (512)

TRACE = False              # test harness can flip this for profiling
_LAST_RESULT = {}          # test harness introspection (exec_time_ns, trace)

_CACHE = {}


def _schedule(beta_max, beta_min):
    """Cosine-squared schedule constants, in float32 to match the reference."""
    f32 = np.float32
    softplus = lambda x: f32(np.logaddexp(f32(0.0), f32(x)))
    c_start = softplus(beta_max)
    c_end = softplus(beta_min)
    a = f32(c_start - c_end)
    b = f32(np.pi / (2.0 * T_TOTAL))
    c = c_end

    def F(t):
        t = f32(t)
        return f32(a * np.sin(f32(2.0 * b * t)) / (4.0 * b) + a * t / f32(2.0) + c * t)

    steps = np.arange(NUM_STEPS, dtype=np.float32)
    dF = np.array(
        [F((s + 1.0) * DT) - F(s * DT) for s in steps], dtype=np.float32
    )
    alpha = (f32(1.0) - np.exp(f32(-2.0) * dF)).astype(np.float32)
    kappa = ((ETA * (1.0 - np.sqrt(1.0 - alpha))) ** 2 / alpha).astype(np.float32)
    ts = (steps * f32(DT)).astype(np.float32)
    return ts, alpha, kappa


def _build(ts, alpha, kappa):
    """Build + compile the per-core Bass graph. Returns the Bacc object."""
    import concourse.bacc as bacc
    import concourse.mybir as mybir
    import concourse.tile as tile

    f32 = mybir.dt.float32
    bf16 = mybir.dt.bfloat16
    Alu = mybir.AluOpType
    Act = mybir.ActivationFunctionType

    nc = bacc.Bacc("TRN2", target_bir_lowering=False, debug=False,
                   num_devices=N_CORES)

    MT = HID // 128  # 4 m-tiles

    # DRAM parameters (per-core shards; weights replicated).
    noises_d = nc.dram_tensor("noises", [NUM_STEPS, 128, CW], f32,
                              kind="ExternalInput")
    x0_d = nc.dram_tensor("x0", [128, CW], f32, kind="ExternalInput")
    r_d = nc.dram_tensor("r", [NCH, R_DIM, CW], bf16, kind="ExternalInput")
    w1r_d = nc.dram_tensor("w1r", [R_DIM, HID], bf16, kind="ExternalInput")
    w1x2_d = nc.dram_tensor("w1x2", [128, HID], bf16, kind="ExternalInput")
    w2_d = nc.dram_tensor("w2", [128, MT, Z_DIM], bf16, kind="ExternalInput")
    b1t_d = nc.dram_tensor("b1t", [128, MT, NUM_STEPS], f32,
                           kind="ExternalInput")
    b2t_d = nc.dram_tensor("b2t", [128, NUM_STEPS], f32, kind="ExternalInput")
    muneg_d = nc.dram_tensor("muneg", [128, 1], f32, kind="ExternalInput")

    xt_d = nc.dram_tensor("xt", [NUM_STEPS + 1, 128, CW], f32,
                          kind="ExternalOutput")
    lw_d = nc.dram_tensor("lw", [NCH, CW], f32, kind="ExternalOutput")
    nab_d = nc.dram_tensor("nab", [128, CW], f32, kind="ExternalOutput")

    with tile.TileContext(nc) as tc:
        with (
            tc.tile_pool(name="const", bufs=1) as cpool,
            tc.tile_pool(name="xstp", bufs=3) as xpool,
            tc.tile_pool(name="xbfp", bufs=3) as xbpool,
            tc.tile_pool(name="noise", bufs=6) as npool,
            tc.tile_pool(name="work", bufs=3) as wpool,
            tc.tile_pool(name="hbuf", bufs=2) as hpool,
            tc.tile_pool(name="hps", bufs=3, space="PSUM") as hps_pool,
            tc.tile_pool(name="sps", bufs=1, space="PSUM") as sps_pool,
        ):
            # --- constants / weights ---
            w1r = cpool.tile([R_DIM, HID], bf16)
            nc.sync.dma_start(out=w1r[:], in_=w1r_d[:])
            w1x2 = cpool.tile([128, HID], bf16)
            nc.sync.dma_start(out=w1x2[:], in_=w1x2_d[:])
            w2 = cpool.tile([128, MT, Z_DIM], bf16)
            nc.sync.dma_start(out=w2[:], in_=w2_d[:])
            b1t = cpool.tile([128, MT, NUM_STEPS], f32)
            nc.sync.dma_start(out=b1t[:], in_=b1t_d[:])
            b2t = cpool.tile([128, NUM_STEPS], f32)
            nc.sync.dma_start(out=b2t[:], in_=b2t_d[:])
            muneg = cpool.tile([128, 1], f32)
            nc.sync.dma_start(out=muneg[:], in_=muneg_d[:])
            rbf = [cpool.tile([R_DIM, CW], bf16, name=f"rbf{c}")
                   for c in range(NCH)]
            for c in range(NCH):
                nc.sync.dma_start(out=rbf[c][:], in_=r_d[c])

            # --- x state: fp32 (xst) + bf16 matmul copy (xbf) ---
            xst = xpool.tile([128, CW], f32, tag="xst")
            nc.sync.dma_start(out=xst[:], in_=x0_d[:])
            xbf = xbpool.tile([128, CW], bf16, tag="xbf")
            nc.vector.tensor_copy(xbf[:], xst[:])
            nc.sync.dma_start(out=xt_d[0], in_=xst[:])

            # log-weight accumulator (both chunks merged on partitions)
            lwacc = cpool.tile([128, CW], f32, name="lwacc")
            nc.gpsimd.memset(lwacc[:], 0.0)

            # --- main loop ---
            for s in range(NUM_STEPS):
                cs = float(1.0 - np.sqrt(1.0 - alpha[s]))
                sa = float(ETA * np.sqrt(alpha[s]))
                cs1 = float(np.float32(1.0) - np.float32(cs))
                twosa = float(2.0 * np.sqrt(alpha[s]))

                n_t = npool.tile([128, CW], f32, tag="noise")
                nc.sync.dma_start(out=n_t[:], in_=noises_d[s])

                # g = (1-cs)*x + sa*n: both inputs exist at step start, so
                # this runs during MM1/tanh, off the critical chain.
                g_t = wpool.tile([128, CW], f32, tag="g")
                nc.vector.tensor_scalar(g_t[:], n_t[:], sa, None, Alu.mult)
                nc.vector.scalar_tensor_tensor(g_t[:], xst[:], cs1, g_t[:],
                                               Alu.mult, Alu.add)

                # the r-half of MM1 doesn't depend on x: issue it first so the
                # PE has ready work while step s-1's tail chain completes
                # (keeps the HAM clock gate warm).
                h = hpool.tile([128, MT, 2 * CW], bf16, tag="h")
                pss = []
                for m in range(MT):
                    ms = slice(m * 128, (m + 1) * 128)
                    ps = hps_pool.tile([128, 2 * CW], f32, tag="hps")
                    pss.append(ps)
                    nc.tensor.matmul(ps[:, 0:CW], w1r[:, ms], rbf[0][:],
                                     start=True, stop=False)
                    nc.tensor.matmul(ps[:, CW:2 * CW], w1r[:, ms], rbf[1][:],
                                     start=True, stop=False)

                for m in range(MT):
                    ms = slice(m * 128, (m + 1) * 128)
                    ps = pss[m]
                    nc.tensor.matmul(ps[:, 0:CW], w1x2[0:64, ms],
                                     xbf[0:64, :], start=False, stop=True)
                    nc.tensor.matmul(ps[:, CW:2 * CW], w1x2[64:128, ms],
                                     xbf[64:128, :], start=False, stop=True)
                    # tanh(h_pre + b1 + t_s * W1[192]) fused via bias table
                    nc.scalar.activation(h[:, m, :], ps[:], Act.Tanh,
                                         bias=b1t[:, m, s:s + 1], scale=1.0)

                # MM2: chunk 0 -> partitions 0:64, chunk 1 -> 64:128
                sps = sps_pool.tile([128, CW], f32, tag="sps")
                for k in range(MT):
                    nc.tensor.matmul(sps[0:64, :], w2[:, k, :],
                                     h[:, k, 0:CW],
                                     start=(k == 0), stop=(k == MT - 1))
                    nc.tensor.matmul(sps[64:128, :], w2[:, k, :],
                                     h[:, k, CW:2 * CW],
                                     start=(k == 0), stop=(k == MT - 1))
                # sc2 = 2*cs*(score + b2) on ScalarE (idle once the step's
                # tanh burst is done), overlapping VectorE's g/nsa work
                sc2 = wpool.tile([128, CW], f32, tag="sc")
                nc.scalar.activation(sc2[:], sps[:], Act.Identity,
                                     bias=b2t[:, s:s + 1], scale=2.0 * cs)

                # PE "heater": a throwaway fp32 matmul pinned (via its sc2
                # read) into the tail-chain window, so the HAM clock gate
                # never sees a long TensorE idle and the array stays at
                # 2.4 GHz. Output is never read.
                ht1 = sps_pool.tile([1, CW], f32, tag="lwps")
                nc.tensor.matmul(ht1[:], muneg[:, 0:1], sc2[:],
                                 start=True, stop=True)

                # x_new = sc2 + g — one op on the critical path (bf16 out for
                # next step's x-matmuls); the fp32 twin runs off-path.
                xbf = xbpool.tile([128, CW], bf16, tag="xbf")
                nc.vector.tensor_tensor(xbf[:], sc2[:], g_t[:], Alu.add)
                xst = xpool.tile([128, CW], f32, tag="xst")
                nc.vector.tensor_tensor(xst[:], sc2[:], g_t[:], Alu.add)
                nc.sync.dma_start(out=xt_d[s + 1], in_=xst[:])

                # log-weight increment (off the critical path):
                #   -2ka*s^2 - 2sqrt(ka)*s*n == -1/(2a) * sc2*(sc2+2sqrt(a)n)
                nka = float(-1.0 / (2.0 * alpha[s]))
                w_t = wpool.tile([128, CW], f32, tag="w")
                nc.vector.scalar_tensor_tensor(w_t[:], n_t[:], twosa,
                                               sc2[:], Alu.mult, Alu.add)
                p_t = wpool.tile([128, CW], f32, tag="p")
                nc.vector.tensor_tensor(p_t[:], sc2[:], w_t[:], Alu.mult)
                nc.vector.scalar_tensor_tensor(lwacc[:], p_t[:], nka,
                                               lwacc[:], Alu.mult, Alu.add)

            # --- epilogue: log_weights and nabla_g ---
            # terminal log-weight 0.5*(x^2 - (x-mu)^2), then reduce over the
            # 64 feature rows of each chunk via a ones-matmul (redtab[:, S]
            # holds the 0.5).
            xm = wpool.tile([128, CW], f32, tag="w")
            nc.scalar.activation(xm[:], xst[:], Act.Square,
                                 bias=muneg[:, 0:1], scale=1.0)
            xsq = wpool.tile([128, CW], f32, tag="m")
            nc.scalar.activation(xsq[:], xst[:], Act.Square, bias=0.0,
                                 scale=1.0)
            td = wpool.tile([128, CW], f32, tag="p")
            nc.vector.tensor_tensor(td[:], xsq[:], xm[:], Alu.subtract)
            tot = wpool.tile([128, CW], f32, tag="sc")
            nc.vector.scalar_tensor_tensor(tot[:], td[:], 0.5, lwacc[:],
                                           Alu.mult, Alu.add)
            ones = wpool.tile([128, 1], f32, tag="ones")
            nc.gpsimd.memset(ones[:], 1.0)
            for c in range(NCH):
                lw_ps = sps_pool.tile([1, CW], f32, tag="lwps")
                nc.tensor.matmul(lw_ps[:], ones[c * 64:(c + 1) * 64, 0:1],
                                 tot[c * 64:(c + 1) * 64, :],
                                 start=True, stop=True)
                lw_sb = wpool.tile([1, CW], f32, tag="lwsb")
                nc.vector.tensor_copy(lw_sb[:], lw_ps[:])
                nc.sync.dma_start(out=lw_d[c:c + 1, :], in_=lw_sb[:])

            # nabla_g = (x - mu) - x  (== -mu up to rounding, as reference)
            d1 = wpool.tile([128, CW], f32, tag="m")
            nc.scalar.activation(d1[:], xst[:], Act.Identity,
                                 bias=muneg[:, 0:1], scale=1.0)
            nab = wpool.tile([128, CW], f32, tag="sc")
            nc.vector.tensor_tensor(nab[:], d1[:], xst[:], Alu.subtract)
            nc.sync.dma_start(out=nab_d[:], in_=nab[:])

    nc.compile()
    return nc


def kernel(r, noises, x0, W1, b1, W2, b2, beta_max, beta_min, target_mu):
    import ml_dtypes
    from concourse.bass_utils import run_bass_kernel_spmd

    bf = ml_dtypes.bfloat16
    r = np.asarray(r, np.float32)
    noises = np.asarray(noises, np.float32)
    x0 = np.asarray(x0, np.float32)
    W1 = np.asarray(W1, np.float32)
    b1 = np.asarray(b1, np.float32)
    W2 = np.asarray(W2, np.float32)
    b2 = np.asarray(b2, np.float32)
    bmax = float(np.asarray(beta_max))
    bmin = float(np.asarray(beta_min))
    mu = np.asarray(target_mu, np.float32)

    ts, alpha, kappa = _schedule(bmax, bmin)

    key = (bmax, bmin)
    if key not in _CACHE:
        _CACHE[key] = _build(ts, alpha, kappa)
    nc = _CACHE[key]

    # Host-side shard prep (pure data movement + the schedule-bias folds).
    r_flat = r.reshape(B, R_DIM)
    # bias table: b1 + t_s * W1[last row] -> [128, 4, S]
    b1t = (b1[None, :] + ts[:, None] * W1[Z_DIM + R_DIM, :]).astype(np.float32)
    b1t = np.ascontiguousarray(
        b1t.reshape(NUM_STEPS, HID // 128, 128).transpose(2, 1, 0))
    cs_all = (1.0 - np.sqrt(1.0 - alpha)).astype(np.float32)  # [S]
    b2t_half = (2.0 * cs_all[None, :] * b2[:, None]).astype(np.float32)
    b2t_in = np.ascontiguousarray(np.concatenate([b2t_half, b2t_half], 0))
    muneg_in = np.ascontiguousarray(
        np.concatenate([-mu, -mu], 0)[:, None].astype(np.float32))
    w1r_in = np.ascontiguousarray(W1[Z_DIM:Z_DIM + R_DIM]).astype(bf)
    w1x2_in = np.ascontiguousarray(
        np.concatenate([W1[:Z_DIM], W1[:Z_DIM]], 0)).astype(bf)
    w2_in = np.ascontiguousarray(
        W2.reshape(HID // 128, 128, Z_DIM).transpose(1, 0, 2)).astype(bf)

    in_maps = []
    for i in range(N_CORES):
        b0, b1i = i * BS, (i + 1) * BS
        nsh = noises[b0:b1i]            # [BS, S, D]
        nT = np.ascontiguousarray(
            nsh.reshape(NCH, CW, NUM_STEPS, Z_DIM).transpose(2, 0, 3, 1)
        ).reshape(NUM_STEPS, 128, CW)
        x0T = np.ascontiguousarray(
            x0[b0:b1i].reshape(NCH, CW, Z_DIM).transpose(0, 2, 1)
        ).reshape(128, CW)
        rT = np.ascontiguousarray(
            r_flat[b0:b1i].reshape(NCH, CW, R_DIM).transpose(0, 2, 1)
        ).astype(bf)
        in_maps.append({
            "noises": nT, "x0": x0T, "r": rT,
            "w1r": w1r_in, "w1x2": w1x2_in, "w2": w2_in, "b1t": b1t,
            "b2t": b2t_in, "muneg": muneg_in,
        })

    res = run_bass_kernel_spmd(nc, in_maps, core_ids=list(range(N_CORES)),
                               trace=TRACE)
    _LAST_RESULT.clear()
    _LAST_RESULT["exec_time_ns"] = res.exec_time_ns
    _LAST_RESULT["trace"] = (res.instructions_and_trace[1]
                             if res.instructions_and_trace else None)
    _LAST_RESULT["profile_json"] = res.profile_json

    x_t = np.empty((B, NUM_STEPS + 1, Z_DIM), np.float32)
    log_w = np.empty((B,), np.float32)
    nab = np.empty((B, Z_DIM), np.float32)
    for i in range(N_CORES):
        b0, b1i = i * BS, (i + 1) * BS
        out = res.results[i]
        # [S+1, 128, CW] -> [NCH, CW, S+1, D]
        x_t[b0:b1i] = out["xt"].reshape(
            NUM_STEPS + 1, NCH, Z_DIM, CW).transpose(1, 3, 0, 2).reshape(
            BS, NUM_STEPS + 1, Z_DIM)
        log_w[b0:b1i] = out["lw"].reshape(BS)
        nab[b0:b1i] = out["nab"].reshape(NCH, Z_DIM, CW).transpose(
            0, 2, 1).reshape(BS, Z_DIM)
    return x_t, log_w, nab
